# revision 24
# baseline (speedup 1.0000x reference)
"""Deformable Conv2d (B=8, C=O=64, H=W=128, K=3) on 8 Trainium2 NeuronCores.

Data-parallel over batch: core b handles batch b.

Per-core algorithm (all on device):
  1. Pad x by 2 (zeros) -> x_pad2 [132,132] so every clamped bilinear corner
     reads an exact zero (reference's out-of-bounds corners contribute 0).
  2. Build P2 in DRAM: token t=(y*132+x) holds [x_pad2[y,x,:64c], x_pad2[y+1,x,:64c]]
     in bf16 (256B). A 512B gather elem starting at token t covers the full
     2x2 corner patch (x and x+1 columns, y and y+1 rows) via elem_step=128.
     The row-pair zip happens in SBUF (z tile) so the P2 DMA writes 256B runs.
  3. Index math runs TWICE in two layouts:
     - weight layout [p=w, j=h]: fractional weights for the 4 corners
       (pair-packed bf16 wpair for the multiply stage), same as before.
     - idx layout [p=16r+b, F] <-> px = r*2048 + F*16 + b: the i16 gather
       indices come out so that the wrap to the gather's [i%16, i//16]
       wrapped layout is 8 contiguous-run DMAs per kp (256B runs) plus 7
       per-kp replication DMAs, instead of a 2-byte-granule scatter.
       Base tables (h(p,F)+ky, w(p,F)+kx) are host-precomputed (btab input).
  4. dma_gather (px-major): dst[i%128, i//16..] patches; multiply by
     corner weights (pair-packed broadcast b operand, bf16 2x mode);
     adds reduce 4 corners -> sampled [128px, kp-pair, c].
  5. PE-transpose sampled to [kp-pair*c, px]; 5 accumulating matmuls
     (K=128 = 2 kp x 64c; 5th block half zero-padded) -> out[o, px]; ACT bias.
"""

import numpy as np
import ml_dtypes

C = 64
O = 64
H = 128
W = 128
KP = 9
PX = H * W                    # 16384
W2 = 132                      # padded-by-2 width
NTOK = W2 * W2                # 17424
NB = 138                      # x_sb col blocks (138*128 = 17664 >= NTOK+132+...)
NELEM = NTOK - 1              # gather index bound (reads tokens idx, idx+1)
MAGIC = 12582912.0            # 3 * 2**22, f32 round-to-nearest magic
CHUNK = 4096                  # px per gather chunk
NCHUNK = PX // CHUNK          # 4
NJC = CHUNK // 128            # 32 j-blocks per chunk
NJ = PX // 128                # 128 j-blocks total
N_CORES = 8

bf16 = ml_dtypes.bfloat16

_CACHE = {}


def _build_program(repeat=1):
    import concourse.bacc as bacc
    import concourse.bass as bass
    import concourse.mybir as mybir
    import concourse.tile as tile
    from concourse import library_config
    from concourse.masks import make_identity

    f32 = mybir.dt.float32
    bff = mybir.dt.bfloat16
    i16 = mybir.dt.int16
    AF = mybir.ActivationFunctionType
    OP = mybir.AluOpType

    nc = bacc.Bacc("TRN2", target_bir_lowering=False, debug=False)

    xin = nc.dram_tensor("xin", [C, PX], f32, kind="ExternalInput")
    offin = nc.dram_tensor("offin", [2 * KP, PX], f32, kind="ExternalInput")
    wT = nc.dram_tensor("wT", [5, 128, O], bff, kind="ExternalInput")
    bin_ = nc.dram_tensor("bin", [O, 1], f32, kind="ExternalInput")
    btab = nc.dram_tensor("btab", [6, 128, 128], f32, kind="ExternalInput")
    out = nc.dram_tensor("out", [O, PX], f32, kind="ExternalOutput")
    P2 = nc.dram_tensor("P2", [NTOK * 128], bff)

    with tile.TileContext(nc) as tc:
        with (
            tc.tile_pool(name="const", bufs=1) as cpool,
            tc.tile_pool(name="main", bufs=1) as mpool,
            tc.tile_pool(name="wtmp", bufs=2) as wpool,
            tc.tile_pool(name="psA", bufs=2, space="PSUM") as psA,
            tc.tile_pool(name="psB", bufs=2, space="PSUM") as psB,
        ):
            # ---------------- constants ----------------
            ident_bf = cpool.tile([128, 128], bff)
            make_identity(nc, ident_bf[:])
            ident_f = cpool.tile([128, 128], f32)
            make_identity(nc, ident_f[:])

            wT_sb = cpool.tile([128, 5 * O], bff)
            nc.sync.dma_start(
                wT_sb[:],
                bass.AP(wT, 0, [[O, 128], [128 * O, 5], [1, O]]),
            )
            bias_sb = cpool.tile([O, 1], f32)
            nc.sync.dma_start(bias_sb[:], bin_.ap())
            btab_sb = cpool.tile([128, 6, 128], f32)
            nc.sync.dma_start(
                btab_sb[:],
                bass.AP(btab, 0, [[128, 128], [128 * 128, 6], [1, 128]]),
            )

            # basex (per-partition w) variants for the WEIGHT layout
            bx_i = cpool.tile([128, 1], mybir.dt.int32)
            nc.gpsimd.iota(bx_i[:], pattern=[[1, 1]], base=0, channel_multiplier=1)
            bx_f = cpool.tile([128, 1], f32)
            nc.vector.tensor_copy(out=bx_f[:], in_=bx_i[:])
            bx = []
            for kxi in range(3):
                t = cpool.tile([128, 1], f32, tag=f"bx{kxi}")
                nc.vector.tensor_scalar(
                    out=t[:], in0=bx_f[:], scalar1=float(kxi), scalar2=None,
                    op0=OP.add,
                )
                bx.append(t)
            by_i = cpool.tile([128, 128], mybir.dt.int32)
            nc.gpsimd.iota(by_i[:], pattern=[[1, 128]], base=0, channel_multiplier=0)
            by_f = cpool.tile([128, 128], f32)
            nc.vector.tensor_copy(out=by_f[:], in_=by_i[:])
            by = []
            for kyi in range(3):
                t = cpool.tile([128, 128], f32, tag=f"by{kyi}")
                nc.vector.tensor_scalar(
                    out=t[:], in0=by_f[:], scalar1=float(kyi), scalar2=None,
                    op0=OP.add,
                )
                by.append(t)

            nc.gpsimd.load_library(library_config.mlp)

            # tiles shared by prologue + main loop
            idx_wr = mpool.tile([128, KP, PX // 16], i16, tag="idxwr")
            wpair = mpool.tile([128, KP * 4 * NJ * 2], bff, tag="wpair")
            wpv = wpair[:].rearrange(
                "p (k q j e) -> p k q j e", k=KP, q=4, j=NJ
            )

            with tc.tile_pool(name="prep", bufs=1) as ppool:
                off_px = ppool.tile([128, 2 * KP, NJ], f32, tag="offpx")

                # ---------------- offsets load (parallel with x chain) --
                off_f = ppool.tile([2 * KP, PX], f32, tag="offf")
                nc.sync.dma_start(off_f[:], offin.ap())

                # ---------------- x -> P2 ----------------
                # z token t holds [x_pad2[t] | x_pad2[t+132]] (256B): both
                # halves come from xbar transposes of x_sb straight into
                # the column halves of z; one P2 DMA per half-slab then
                # writes 256B runs.
                with tc.tile_pool(name="prepx", bufs=1) as xpool:
                    with tc.tile_pool(name="prepxs", bufs=1) as xspool:
                        x_sb = xspool.tile([C, NB * 128], bff, tag="xsb")
                        # zero only the padding bands (borders + tail)
                        nc.gpsimd.memset(x_sb[:, 0:262], 0.0)
                        nc.gpsimd.memset(
                            x_sb[:, 262:262 + 129 * 132].rearrange(
                                "c (a b) -> c a b", a=129
                            )[:, :, 0:4],
                            0.0,
                        )
                        nc.gpsimd.memset(x_sb[:, 17158:NB * 128], 0.0)
                        interior = x_sb[:, :NTOK].rearrange(
                            "c (a b) -> c a b", a=W2
                        )[:, 2:130, 2:130]
                        nc.gpsimd.dma_start(
                            out=interior,
                            in_=xin.ap().rearrange("c (h w) -> c h w", h=H),
                        )
                        for hb in range(2):
                            z = xpool.tile([128, 68, 128], bff, tag="zzip")
                            nc.sync.dma_start_transpose(
                                z[:, :, 0:C],
                                x_sb[:, hb * 8704:(hb + 1) * 8704],
                            )
                            nc.sync.dma_start_transpose(
                                z[:, :, C:128],
                                x_sb[:, 132 + hb * 8704:132 + (hb + 1) * 8704],
                            )
                            nc.sync.dma_start(
                                bass.AP(
                                    P2, hb * 68 * 128 * 128,
                                    [[128, 128], [128 * 128, 68], [1, 128]],
                                ),
                                z[:],
                            )

                # ---------------- idx-layout offsets + indices ----------
                # slot (p=16r+b, F) <-> px = r*2048 + F*16 + b
                off_v = off_f[:].rearrange(
                    "c (r m b) -> c r m b", r=8, m=128
                )
                # stage TWO F-columns per PE transpose: F-even channels in
                # partitions 0:18, F-odd in partitions 32:50 (PE matmul rhs
                # allows only one free dim, so stage contiguous first)
                offI = ppool.tile([128, 2 * KP, 128], f32, tag="offI")
                for g in range(8):
                    stg = ppool.tile([64, 8 * 128], f32, tag="stgI")
                    for par in range(2):
                        nc.scalar.copy(
                            out=stg[32 * par:32 * par + 2 * KP, :].rearrange(
                                "c (m r b) -> c m r b", m=8, r=8
                            ),
                            in_=off_v[
                                :, :, g * 16 + par:(g + 1) * 16:2, :
                            ].rearrange("c r m b -> c m r b"),
                        )
                    ps = psA.tile([128, 8 * 50], f32, tag="offpsI")
                    for ff in range(8):
                        nc.tensor.transpose(
                            out=ps[:, ff * 50:(ff + 1) * 50],
                            in_=stg[0:50, ff * 128:(ff + 1) * 128],
                            identity=ident_f[0:50, 0:50],
                        )
                    psv = ps[:].rearrange("p (a b) -> p a b", a=8)
                    for par in range(2):
                        nc.scalar.copy(
                            out=offI[:, :, g * 16 + par:(g + 1) * 16:2],
                            in_=psv[:, :, 32 * par:32 * par + 2 * KP].rearrange(
                                "p a c -> p c a"
                            ),
                        )

                idxI = ppool.tile([128, KP, 128], i16, tag="idxI")
                for k in range(KP):
                    kyi, kxi = k // 3, k % 3
                    # btab_y is pre-biased by -0.5 (host side)
                    y0 = wpool.tile([128, 128], f32, tag="y0I")
                    nc.vector.tensor_tensor(
                        out=y0[:], in0=offI[:, 2 * k, :],
                        in1=btab_sb[:, kyi, :], op=OP.add,
                    )
                    nc.vector.tensor_scalar(
                        out=y0[:], in0=y0[:], scalar1=MAGIC, scalar2=MAGIC,
                        op0=OP.add, op1=OP.subtract,
                    )
                    nc.vector.tensor_scalar(
                        out=y0[:], in0=y0[:], scalar1=-1.0, scalar2=129.0,
                        op0=OP.max, op1=OP.min,
                    )
                    iy = wpool.tile([128, 128], f32, tag="iyI")
                    nc.vector.tensor_scalar(
                        out=iy[:], in0=y0[:], scalar1=132.0, scalar2=133.0,
                        op0=OP.mult, op1=OP.add,
                    )
                    x0 = wpool.tile([128, 128], f32, tag="x0I")
                    nc.vector.tensor_tensor(
                        out=x0[:], in0=offI[:, 2 * k + 1, :],
                        in1=btab_sb[:, 3 + kxi, :], op=OP.add,
                    )
                    nc.vector.tensor_scalar(
                        out=x0[:], in0=x0[:], scalar1=MAGIC, scalar2=MAGIC,
                        op0=OP.add, op1=OP.subtract,
                    )
                    nc.vector.tensor_scalar(
                        out=x0[:], in0=x0[:], scalar1=-1.0, scalar2=129.0,
                        op0=OP.max, op1=OP.min,
                    )
                    idxf = wpool.tile([128, 128], f32, tag="idxfI")
                    nc.vector.tensor_tensor(
                        out=idxf[:], in0=iy[:], in1=x0[:], op=OP.add
                    )
                    nc.vector.tensor_copy(out=idxI[:, k, :], in_=idxf[:])


                # wrap per kyi-group of 3 kp: 8 scatter DMAs (256B runs) +
                # log2 replication to the other 7 partition groups
                for kg in range(3):
                    ks = slice(3 * kg, 3 * kg + 3)
                    for r in range(8):
                        nc.scalar.dma_start(
                            idx_wr[0:16, ks, r * 128:(r + 1) * 128],
                            idxI[16 * r:16 * (r + 1), ks, :],
                        )
                    nc.scalar.dma_start(idx_wr[16:32, ks, :], idx_wr[0:16, ks, :])
                    nc.scalar.dma_start(idx_wr[32:64, ks, :], idx_wr[0:32, ks, :])
                    nc.scalar.dma_start(idx_wr[64:128, ks, :], idx_wr[0:64, ks, :])

                # ---------------- weight-layout offsets ----------------
                for g in range(8):
                    ps = psA.tile([128, 16 * 18], f32, tag="offps")
                    for jj in range(16):
                        j = g * 16 + jj
                        nc.tensor.transpose(
                            out=ps[:, jj * 18:(jj + 1) * 18],
                            in_=off_f[:, j * 128:(j + 1) * 128],
                            identity=ident_f[0:18, 0:18],
                        )
                    nc.scalar.copy(
                        out=off_px[:, :, g * 16:(g + 1) * 16],
                        in_=ps[:].rearrange("p (a b) -> p b a", a=16),
                    )

                # ---------------- corner weights (weight layout) -------
                for k in range(KP):
                    kyi, kxi = k // 3, k % 3
                    oy = off_px[:, 2 * k, :]
                    ox = off_px[:, 2 * k + 1, :]
                    zy = wpool.tile([128, NJ], f32, tag="zy")
                    nc.vector.tensor_tensor(out=zy[:], in0=oy, in1=by[kyi][:], op=OP.add)
                    y0 = wpool.tile([128, NJ], f32, tag="y0")
                    nc.vector.tensor_scalar(
                        out=y0[:], in0=zy[:], scalar1=0.5, scalar2=None, op0=OP.subtract
                    )
                    nc.vector.tensor_scalar(
                        out=y0[:], in0=y0[:], scalar1=MAGIC, scalar2=MAGIC,
                        op0=OP.add, op1=OP.subtract,
                    )
                    fy = wpool.tile([128, NJ], f32, tag="fy")
                    nc.vector.tensor_tensor(out=fy[:], in0=zy[:], in1=y0[:], op=OP.subtract)

                    zx = wpool.tile([128, NJ], f32, tag="zx")
                    nc.vector.tensor_scalar(
                        out=zx[:], in0=ox, scalar1=bx[kxi][:], scalar2=None, op0=OP.add
                    )
                    x0 = wpool.tile([128, NJ], f32, tag="x0")
                    nc.vector.tensor_scalar(
                        out=x0[:], in0=zx[:], scalar1=0.5, scalar2=None, op0=OP.subtract
                    )
                    nc.vector.tensor_scalar(
                        out=x0[:], in0=x0[:], scalar1=MAGIC, scalar2=MAGIC,
                        op0=OP.add, op1=OP.subtract,
                    )
                    fx = wpool.tile([128, NJ], f32, tag="fx")
                    nc.vector.tensor_tensor(out=fx[:], in0=zx[:], in1=x0[:], op=OP.subtract)

                    # corner weights: q order [w00, w10, w01, w11] = (dx,dy)
                    w11 = wpool.tile([128, NJ], f32, tag="w11")
                    nc.vector.tensor_tensor(out=w11[:], in0=fy[:], in1=fx[:], op=OP.mult)
                    w10 = wpool.tile([128, NJ], f32, tag="w10")
                    nc.vector.tensor_tensor(out=w10[:], in0=fy[:], in1=w11[:], op=OP.subtract)
                    w01 = wpool.tile([128, NJ], f32, tag="w01")
                    nc.vector.tensor_tensor(out=w01[:], in0=fx[:], in1=w11[:], op=OP.subtract)
                    omfy = wpool.tile([128, NJ], f32, tag="omfy")
                    nc.vector.tensor_scalar(
                        out=omfy[:], in0=fy[:], scalar1=-1.0, scalar2=1.0,
                        op0=OP.mult, op1=OP.add,
                    )
                    w00 = wpool.tile([128, NJ], f32, tag="w00")
                    nc.vector.tensor_tensor(out=w00[:], in0=omfy[:], in1=w01[:], op=OP.subtract)
                    for q, wq in enumerate([w00, w10, w01, w11]):
                        for e in range(2):
                            nc.vector.tensor_copy(
                                out=wpv[:, k, q, :, e], in_=wq[:]
                            )

            # ---------------- main loop ----------------
            loop_pools = tc.tile_pool(name="gath", bufs=2)
            gpool = loop_pools.__enter__()
            mul_cm = tc.tile_pool(name="mul", bufs=2)
            mulpool = mul_cm.__enter__()
            samp_cm = tc.tile_pool(name="samp", bufs=1)
            spool = samp_cm.__enter__()
            stage_cm = tc.tile_pool(name="stage", bufs=3)
            stpool = stage_cm.__enter__()

            for cch in [cc for _ in range(repeat) for cc in range(NCHUNK)]:
                s_pairs = [
                    spool.tile([128, NJC, 2, 64], bff, tag=f"sp{qq}", name=f"sp{qq}")
                    for qq in range(5)
                ]
                nc.vector.memset(s_pairs[4][:, :, 1, :], 0.0)
                for k in range(KP):
                    gt = gpool.tile([128, NJC, 256], bff, tag="g")
                    nc.gpsimd.dma_gather(
                        out_ap=gt[:],
                        in_ap=bass.AP(P2, 0, [[128, NELEM], [1, 256]]),
                        idxs_ap=idx_wr[:, k, cch * (CHUNK // 16):(cch + 1) * (CHUNK // 16)],
                        num_idxs=CHUNK,
                        num_idxs_reg=CHUNK,
                        elem_size=256,
                        elem_step=128,
                        single_packet=False,
                    )
                    mt = [
                        mulpool.tile([128, NJC, 64], bff, tag=f"m{q}", name=f"m{q}")
                        for q in range(4)
                    ]
                    for q in range(4):
                        b_ap = bass.AP(
                            wpair.tensor,
                            wpair[:].offset + ((k * 4 + q) * NJ + cch * NJC) * 2,
                            [[KP * 4 * NJ * 2, 128], [2, NJC], [0, 32], [1, 2]],
                        )
                        nc.vector.tensor_tensor(
                            out=mt[q][:].rearrange("p j (a e) -> p j a e", a=32),
                            in0=gt[:, :, q * 64:(q + 1) * 64].rearrange(
                                "p j (a e) -> p j a e", a=32
                            ),
                            in1=b_ap,
                            op=OP.mult,
                        )
                    a1 = mulpool.tile([128, NJC, 64], bff, tag="a1")
                    nc.vector.tensor_tensor(out=a1[:], in0=mt[0][:], in1=mt[1][:], op=OP.add)
                    a2 = mulpool.tile([128, NJC, 64], bff, tag="a2")
                    nc.vector.tensor_tensor(out=a2[:], in0=mt[2][:], in1=mt[3][:], op=OP.add)
                    nc.vector.tensor_tensor(
                        out=s_pairs[k // 2][:, :, k % 2, :],
                        in0=a1[:], in1=a2[:], op=OP.add,
                    )

                for sub in range(CHUNK // 512):
                    pout = psB.tile([O, 512], mybir.dt.float32, space="PSUM", tag="pout")
                    for q in range(5):
                        pt = psA.tile([128, 512], bff, space="PSUM", tag="pt")
                        for jj in range(4):
                            j = sub * 4 + jj
                            nc.tensor.transpose(
                                out=pt[:, jj * 128:(jj + 1) * 128],
                                in_=s_pairs[q][:, j, :, :].rearrange("p a b -> p (a b)"),
                                identity=ident_bf[:],
                            )
                        st = stpool.tile([128, 512], bff, tag="st")
                        nc.scalar.copy(out=st[:], in_=pt[:])
                        nc.tensor.matmul(
                            out=pout[:],
                            lhsT=wT_sb[:, q * O:(q + 1) * O],
                            rhs=st[:],
                            start=(q == 0),
                            stop=(q == 4),
                        )
                    ob = stpool.tile([O, 512], mybir.dt.float32, tag="ob")
                    nc.scalar.activation(
                        out=ob[:], in_=pout[:], func=AF.Identity,
                        bias=bias_sb[:], scale=1.0,
                    )
                    nc.sync.dma_start(
                        out.ap()[:, cch * CHUNK + sub * 512: cch * CHUNK + (sub + 1) * 512],
                        ob[:],
                    )
            stage_cm.__exit__(None, None, None)
            samp_cm.__exit__(None, None, None)
            mul_cm.__exit__(None, None, None)
            loop_pools.__exit__(None, None, None)

    nc.compile()
    return nc


def _get_program():
    if "nc" not in _CACHE:
        _CACHE["nc"] = _build_program()
    return _CACHE["nc"]


def _make_btab():
    p = np.arange(128)
    F = np.arange(128)
    hh = 16 * (p[:, None] // 16) + F[None, :] // 8     # h(p, F)
    ww = 16 * (F[None, :] % 8) + (p[:, None] % 16)     # w(p, F)
    btab = np.zeros((6, 128, 128), dtype=np.float32)
    for kyi in range(3):
        btab[kyi] = hh + kyi - 0.5
    for kxi in range(3):
        btab[3 + kxi] = ww + kxi - 0.5
    return btab


def kernel(x, offset, weight, bias):
    import os
    from concourse.bass_utils import run_bass_kernel_spmd

    x = np.asarray(x, dtype=np.float32)
    offset = np.asarray(offset, dtype=np.float32)
    weight = np.asarray(weight, dtype=np.float32)
    bias = np.asarray(bias, dtype=np.float32)
    B = x.shape[0]
    assert B == N_CORES

    w3 = weight.reshape(O, C, KP)
    wTn = np.zeros((5, 128, O), dtype=bf16)
    for q in range(5):
        for L in range(2):
            kp = 2 * q + L
            if kp < KP:
                wTn[q, L * 64:(L + 1) * 64, :] = (
                    w3[:, :, kp].T.astype(bf16)
                )
    bias_n = bias.reshape(O, 1).astype(np.float32)
    btab = _make_btab()

    in_maps = []
    for b in range(B):
        in_maps.append({
            "xin": x[b].reshape(C, PX),
            "offin": offset[b].reshape(2 * KP, PX),
            "wT": wTn,
            "bin": bias_n,
            "btab": btab,
        })

    nc = _get_program()
    trace = os.environ.get("DC_TRACE") == "1"
    res = run_bass_kernel_spmd(
        nc, in_maps, list(range(N_CORES)),
        trace=trace, tmpdir=os.environ.get("DC_TRACE_DIR"),
    )
    if res.exec_time_ns is not None:
        _CACHE["exec_time_ns"] = res.exec_time_ns
    outs = [res.results[b]["out"].reshape(O, H, W) for b in range(B)]
    return np.stack(outs, axis=0).astype(np.float32)
